# revision 6
# baseline (speedup 1.0000x reference)
"""Trainium2 Bass kernel for per-sample softmax-modulated 3x3 conv.

ref: out[b] = conv2d(inp[b], softmax(action_out[b]).reshape(3,3) * conv_weight,
                     padding=SAME)  with inp [64,256,64,64], conv_weight [256,256,3,3].

Strategy: data-parallel over 8 NeuronCores (8 samples per core). On each core
the conv is 9 "shifted" matmuls per (ci_blk, co_blk) pair: for tap (dy,dx),
out[co, h, w] += (mask[b,t] * W_t)[co, ci] @ xpad[ci, h+dy, w+dx], accumulated
in PSUM over the 18 (ci_blk, tap) combinations. The softmax tap mask is folded
into the stationary weight tiles once per sample by the vector engine.
Matmuls run as float32r (fp32 storage, full-rate PE streaming at N=512).
"""

import sys

sys.path.insert(0, "/opt/trn_rl_repo")

import numpy as np

B, C, H, W = 64, 256, 64, 64
NCORES = 8
BPC = B // NCORES          # samples per core
HW = H * W                 # 4096
PW = W + 2                 # padded row width 66
PHW = (H + 2) * PW         # padded image size 4356
WCOLS = 9 * 2 * C          # weight tile columns: t*512 + cb*256 + co

_RUNNER = None


def _build_nc():
    import concourse.bacc as bacc
    import concourse.mybir as mybir
    from concourse.tile import TileContext

    f32 = mybir.dt.float32
    f32r = mybir.dt.float32r
    MUL = mybir.AluOpType.mult

    nc = bacc.Bacc("TRN2", target_bir_lowering=False)
    x = nc.declare_dram_parameter("x", [BPC, C, HW], f32r, isOutput=False)
    a = nc.declare_dram_parameter("a", [1, BPC * 9], f32, isOutput=False)
    w = nc.declare_dram_parameter("w", [128, WCOLS], f32r, isOutput=False)
    y = nc.declare_dram_parameter("y", [BPC, C, HW], f32, isOutput=True)

    with TileContext(nc) as tc:
        with (
            tc.tile_pool(name="const", bufs=1) as cpool,
            tc.tile_pool(name="xp", bufs=1) as xpool,
            tc.tile_pool(name="wsp", bufs=1) as wpool,
            tc.tile_pool(name="psum", bufs=8, space="PSUM") as ppool,
            tc.tile_pool(name="stg", bufs=4) as spool,
        ):
            wT = cpool.tile([128, WCOLS], f32r, tag="wT")
            nc.sync.dma_start(out=wT[:], in_=w[:])

            a_sb = cpool.tile([1, BPC * 9], f32, tag="a_sb")
            nc.sync.dma_start(out=a_sb[:], in_=a[:])
            ones = cpool.tile([1, 128], f32, tag="ones")
            nc.vector.memset(ones[:], 1.0)

            # broadcast raw action logits to all 128 partitions via K=1 matmul
            pb = ppool.tile([128, BPC * 9], f32, tag="acc")
            nc.tensor.matmul(pb[:], ones[:], a_sb[:], start=True, stop=True)
            exp_a = cpool.tile([128, BPC * 9], f32, tag="exp_a")
            nc.scalar.activation(exp_a[:], pb[:], mybir.ActivationFunctionType.Exp)
            zsum = cpool.tile([128, BPC], f32, tag="zsum")
            nc.vector.reduce_sum(
                zsum[:],
                exp_a[:].rearrange("p (b t) -> p b t", t=9),
                axis=mybir.AxisListType.X,
            )
            invz = cpool.tile([128, BPC], f32, tag="invz")
            nc.vector.reciprocal(invz[:], zsum[:])

            # fixed padded-input tiles: 2 channel blocks x 2 sample parity,
            # zero borders written once, interiors DMA'd per sample
            xpad = [
                [
                    xpool.tile(
                        [128, PHW], f32r, tag=f"xp{cb}_{par}", name=f"xp{cb}_{par}"
                    )
                    for par in range(2)
                ]
                for cb in range(2)
            ]
            for cb in range(2):
                for par in range(2):
                    # memset doesn't accept f32r; zero bits are dtype-agnostic
                    v = xpad[cb][par].bitcast(f32).rearrange("p (r c) -> p r c", c=PW)
                    nc.vector.memset(v[:, 0:1, :], 0.0)
                    nc.vector.memset(v[:, H + 1 : H + 2, :], 0.0)
                    nc.vector.memset(v[:, 1 : H + 1, 0:1], 0.0)
                    nc.vector.memset(v[:, 1 : H + 1, W + 1 : W + 2], 0.0)

            wS = [
                wpool.tile([128, WCOLS], f32r, tag=f"wS{par}", name=f"wS{par}")
                for par in range(2)
            ]

            for b in range(BPC):
                par = b % 2
                wSb = wS[par]
                # fold softmax(action)[b, t] into the stationary weights
                for t in range(9):
                    nc.vector.tensor_scalar(
                        out=wSb[:, t * 512 : (t + 1) * 512],
                        in0=wT[:, t * 512 : (t + 1) * 512],
                        scalar1=exp_a[:, b * 9 + t : b * 9 + t + 1],
                        scalar2=invz[:, b : b + 1],
                        op0=MUL,
                        op1=MUL,
                    )
                for cb in range(2):
                    dst = xpad[cb][par].rearrange("p (r c) -> p r c", c=PW)[
                        :, 1 : H + 1, 1 : W + 1
                    ]
                    src = x[b, cb * 128 : (cb + 1) * 128, :].rearrange(
                        "p (r c) -> p r c", c=W
                    )
                    nc.sync.dma_start(out=dst, in_=src)

                for cob in range(2):
                    accs = [
                        ppool.tile([128, 512], f32, tag="acc", name="acc")
                        for _ in range(8)
                    ]
                    idx = 0
                    for cb in range(2):
                        xv = xpad[cb][par].rearrange("p (r c) -> p r c", c=PW)
                        for t in range(9):
                            dy, dx = divmod(t, 3)
                            off = t * 512 + cb * 256 + cob * 128
                            lhsT = wSb[:, off : off + 128]
                            for pc in range(8):
                                rhs = xv[
                                    :, pc * 8 + dy : pc * 8 + dy + 8, dx : dx + W
                                ]
                                nc.tensor.matmul(
                                    accs[pc][:],
                                    lhsT,
                                    rhs,
                                    start=(idx == 0),
                                    stop=(idx == 17),
                                )
                            idx += 1
                    for pc in range(8):
                        st = spool.tile([128, 512], f32, tag="stg")
                        nc.vector.tensor_copy(st[:], accs[pc][:])
                        nc.sync.dma_start(
                            out=y[
                                b,
                                cob * 128 : (cob + 1) * 128,
                                pc * 512 : (pc + 1) * 512,
                            ],
                            in_=st[:],
                        )

    nc.finalize()
    return nc


class _Runner:
    """Builds the Bass module once and keeps the sharded jit callable warm."""

    def __init__(self):
        import jax
        import concourse.mybir as mybir
        from jax.sharding import Mesh, PartitionSpec
        from jax.experimental.shard_map import shard_map
        from concourse import bass2jax

        bass2jax.install_neuronx_cc_hook()
        self.jax = jax
        nc = _build_nc()
        self.nc = nc

        in_names, out_names, out_avals = [], [], []
        partition_name = (
            nc.partition_id_tensor.name if nc.partition_id_tensor else None
        )
        for alloc in nc.m.functions[0].allocations:
            if not isinstance(alloc, mybir.MemoryLocationSet):
                continue
            name = alloc.memorylocations[0].name
            if alloc.kind == "ExternalInput":
                if name != partition_name:
                    in_names.append(name)
            elif alloc.kind == "ExternalOutput":
                out_names.append(name)
                out_avals.append(
                    jax.core.ShapedArray(
                        tuple(alloc.tensor_shape), mybir.dt.np(alloc.dtype)
                    )
                )
        self.in_names = list(in_names)
        self.out_names = out_names
        self.out_avals = out_avals
        n_params = len(in_names)
        all_in_names = in_names + out_names
        if partition_name is not None:
            all_in_names = all_in_names + [partition_name]

        def _body(*args):
            operands = list(args)
            if partition_name is not None:
                operands.append(bass2jax.partition_id_tensor())
            outs = bass2jax._bass_exec_p.bind(
                *operands,
                out_avals=tuple(out_avals),
                in_names=tuple(all_in_names),
                out_names=tuple(out_names),
                lowering_input_output_aliases=(),
                sim_require_finite=True,
                sim_require_nnan=True,
                nc=nc,
            )
            return tuple(outs)

        devices = jax.devices()[:NCORES]
        self.mesh = Mesh(np.asarray(devices), ("core",))
        in_specs = (PartitionSpec("core"),) * (n_params + len(out_names))
        out_specs = (PartitionSpec("core"),) * len(out_names)
        self.fn = jax.jit(
            shard_map(
                _body,
                mesh=self.mesh,
                in_specs=in_specs,
                out_specs=out_specs,
                check_rep=False,
            ),
            donate_argnums=tuple(
                range(n_params, n_params + len(out_names))
            ),
            keep_unused=True,
        )

    def pack(self, inp, action_out, conv_weight):
        """Full inputs -> concatenated per-core arrays (axis 0 = core-major)."""
        x = np.ascontiguousarray(
            np.asarray(inp, dtype=np.float32).reshape(B, C, HW)
        )  # [64,256,4096]; shard_map slices rows 8i:8i+8 per core
        a = np.asarray(action_out, dtype=np.float32).reshape(NCORES, 1, BPC * 9)
        a = np.ascontiguousarray(a.reshape(NCORES * 1, BPC * 9))
        wt = np.asarray(conv_weight, dtype=np.float32).transpose(1, 2, 3, 0)
        wt = wt.reshape(2, 128, 9, C).transpose(1, 2, 0, 3).reshape(128, WCOLS)
        wrep = np.ascontiguousarray(np.broadcast_to(wt, (NCORES, 128, WCOLS)))
        wrep = wrep.reshape(NCORES * 128, WCOLS)
        return [x, a, wrep]

    def zeros(self):
        return [
            np.zeros((NCORES * av.shape[0], *av.shape[1:]), av.dtype)
            for av in self.out_avals
        ]

    def run(self, packed, zeros=None):
        if zeros is None:
            zeros = self.zeros()
        outs = self.fn(*packed, *zeros)
        y = np.asarray(outs[0])
        return y.reshape(B, C, H, W)


def _get_runner():
    global _RUNNER
    if _RUNNER is None:
        _RUNNER = _Runner()
    return _RUNNER


def kernel(inp, action_out, conv_weight):
    r = _get_runner()
    packed = r.pack(inp, action_out, conv_weight)
    return r.run(packed)


if __name__ == "__main__":
    rng = np.random.default_rng(0)
    inp = rng.standard_normal((B, C, H, W), dtype=np.float32)
    act = rng.standard_normal((B, 9), dtype=np.float32)
    wgt = (rng.random((C, C, 3, 3), dtype=np.float32) - 0.5) * (2.0 / (C * 9))
    out = kernel(inp, act, wgt)
    print("out", out.shape, out.dtype, float(np.abs(out).max()))


# revision 14
# speedup vs baseline: 95.2159x; 95.2159x over previous
"""Trainium2 Bass kernel for per-sample softmax-modulated 3x3 conv.

ref: out[b] = conv2d(inp[b], softmax(action_out[b]).reshape(3,3) * conv_weight,
                     padding=SAME)  with inp [64,256,64,64], conv_weight [256,256,3,3].

Strategy: data-parallel over 8 NeuronCores (8 samples per core). On each core
the conv is 9 "shifted" matmuls per (ci_blk, co_blk) pair: for tap (dy,dx),
out[co, h, w] += (mask[b,t] * W_t)[co, ci] @ xpad[ci, h+dy, w+dx], accumulated
in PSUM over the 18 (ci_blk, tap) combinations. The softmax tap mask is folded
into the stationary weight tiles once per sample by the vector engine.
Matmuls run as float32r (fp32 storage, full-rate PE streaming at N=512).
"""

import sys

sys.path.insert(0, "/opt/trn_rl_repo")

import numpy as np

B, C, H, W = 64, 256, 64, 64
NCORES = 8
BPC = B // NCORES          # samples per core
HW = H * W                 # 4096
PW = W + 2                 # padded row width 66
PHW = (H + 2) * PW         # padded image size 4356
WCOLS = 9 * 2 * C          # weight tile columns: t*512 + cb*256 + co

_RUNNER = None


def _build_nc(repeat=1):
    import contextlib
    import concourse.bacc as bacc
    import concourse.mybir as mybir
    from concourse.tile import TileContext

    f32 = mybir.dt.float32
    f32r = mybir.dt.float32r
    MUL = mybir.AluOpType.mult

    nc = bacc.Bacc("TRN2", target_bir_lowering=False)
    x = nc.declare_dram_parameter("x", [BPC, C, HW], f32r, isOutput=False)
    a = nc.declare_dram_parameter("a", [1, BPC * 9], f32, isOutput=False)
    w = nc.declare_dram_parameter("w", [128, WCOLS], f32r, isOutput=False)
    y = nc.declare_dram_parameter("y", [BPC, C, HW], f32, isOutput=True)

    with TileContext(nc) as tc:
        with (
            tc.tile_pool(name="const", bufs=1) as cpool,
            tc.tile_pool(name="xp", bufs=1) as xpool,
            tc.tile_pool(name="wsp", bufs=1) as wpool,
            tc.tile_pool(name="psum", bufs=8, space="PSUM") as ppool,
            tc.tile_pool(name="stg", bufs=4) as spool,
        ):
            wT = cpool.tile([128, WCOLS], f32r, tag="wT")
            nc.sync.dma_start(out=wT[:], in_=w[:])

            a_sb = cpool.tile([1, BPC * 9], f32, tag="a_sb")
            nc.sync.dma_start(out=a_sb[:], in_=a[:])
            ones = cpool.tile([1, 128], f32, tag="ones")
            nc.vector.memset(ones[:], 1.0)

            # broadcast raw action logits to all 128 partitions via K=1 matmul
            pb = ppool.tile([128, BPC * 9], f32, tag="acc")
            nc.tensor.matmul(pb[:], ones[:], a_sb[:], start=True, stop=True)
            exp_a = cpool.tile([128, BPC * 9], f32, tag="exp_a")
            nc.scalar.activation(exp_a[:], pb[:], mybir.ActivationFunctionType.Exp)
            zsum = cpool.tile([128, BPC], f32, tag="zsum")
            nc.vector.reduce_sum(
                zsum[:],
                exp_a[:].rearrange("p (b t) -> p b t", t=9),
                axis=mybir.AxisListType.X,
            )
            invz = cpool.tile([128, BPC], f32, tag="invz")
            nc.vector.reciprocal(invz[:], zsum[:])

            # fixed padded-input tiles: 2 channel blocks x 2 sample parity,
            # zero borders written once, interiors DMA'd per sample
            xpad = [
                [
                    xpool.tile(
                        [128, PHW], f32r, tag=f"xp{cb}_{par}", name=f"xp{cb}_{par}"
                    )
                    for par in range(2)
                ]
                for cb in range(2)
            ]
            for cb in range(2):
                for par in range(2):
                    # memset doesn't accept f32r; zero bits are dtype-agnostic
                    v = xpad[cb][par].bitcast(f32).rearrange("p (r c) -> p r c", c=PW)
                    nc.vector.memset(v[:, 0:1, :], 0.0)
                    nc.vector.memset(v[:, H + 1 : H + 2, :], 0.0)
                    nc.vector.memset(v[:, 1 : H + 1, 0:1], 0.0)
                    nc.vector.memset(v[:, 1 : H + 1, W + 1 : W + 2], 0.0)

            wS = [
                wpool.tile([128, WCOLS], f32r, tag=f"wS{par}", name=f"wS{par}")
                for par in range(2)
            ]

            loop_ctx = (
                tc.For_i(0, repeat, 1) if repeat > 1 else contextlib.nullcontext()
            )
            with loop_ctx:
              for b in range(BPC):
                par = b % 2
                wSb = wS[par]
                # fold softmax(action)[b, t] into the stationary weights
                for t in range(9):
                    nc.vector.tensor_scalar(
                        out=wSb[:, t * 512 : (t + 1) * 512],
                        in0=wT[:, t * 512 : (t + 1) * 512],
                        scalar1=exp_a[:, b * 9 + t : b * 9 + t + 1],
                        scalar2=invz[:, b : b + 1],
                        op0=MUL,
                        op1=MUL,
                    )
                for cb in range(2):
                    dst = xpad[cb][par].rearrange("p (r c) -> p r c", c=PW)[
                        :, 1 : H + 1, 1 : W + 1
                    ]
                    src = x[b, cb * 128 : (cb + 1) * 128, :].rearrange(
                        "p (r c) -> p r c", c=W
                    )
                    nc.sync.dma_start(out=dst, in_=src)

                for cob in range(2):
                    accs = [
                        ppool.tile([128, 512], f32, tag="acc", name="acc")
                        for _ in range(8)
                    ]
                    idx = 0
                    for cb in range(2):
                        xv = xpad[cb][par].rearrange("p (r c) -> p r c", c=PW)
                        for t in range(9):
                            dy, dx = divmod(t, 3)
                            off = t * 512 + cb * 256 + cob * 128
                            lhsT = wSb[:, off : off + 128]
                            for pc in range(8):
                                rhs = xv[
                                    :, pc * 8 + dy : pc * 8 + dy + 8, dx : dx + W
                                ]
                                nc.tensor.matmul(
                                    accs[pc][:],
                                    lhsT,
                                    rhs,
                                    start=(idx == 0),
                                    stop=(idx == 17),
                                )
                            idx += 1
                    for pc in range(8):
                        st = spool.tile([128, 512], f32, tag="stg")
                        nc.vector.tensor_copy(st[:], accs[pc][:])
                        nc.sync.dma_start(
                            out=y[
                                b,
                                cob * 128 : (cob + 1) * 128,
                                pc * 512 : (pc + 1) * 512,
                            ],
                            in_=st[:],
                        )

    nc.finalize()
    return nc


class _Runner:
    """Builds the Bass module once and keeps the sharded jit callable warm."""

    def __init__(self, repeat=1):
        import jax
        import concourse.mybir as mybir
        from jax.sharding import Mesh, PartitionSpec
        from jax.experimental.shard_map import shard_map
        from concourse import bass2jax

        bass2jax.install_neuronx_cc_hook()
        self.jax = jax
        nc = _build_nc(repeat=repeat)
        self.nc = nc

        in_names, out_names, out_avals = [], [], []
        partition_name = (
            nc.partition_id_tensor.name if nc.partition_id_tensor else None
        )
        for alloc in nc.m.functions[0].allocations:
            if not isinstance(alloc, mybir.MemoryLocationSet):
                continue
            name = alloc.memorylocations[0].name
            if alloc.kind == "ExternalInput":
                if name != partition_name:
                    in_names.append(name)
            elif alloc.kind == "ExternalOutput":
                out_names.append(name)
                out_avals.append(
                    jax.core.ShapedArray(
                        tuple(alloc.tensor_shape), mybir.dt.np(alloc.dtype)
                    )
                )
        self.in_names = list(in_names)
        self.out_names = out_names
        self.out_avals = out_avals
        n_params = len(in_names)
        all_in_names = in_names + out_names
        if partition_name is not None:
            all_in_names = all_in_names + [partition_name]

        def _body(*args):
            operands = list(args)
            if partition_name is not None:
                operands.append(bass2jax.partition_id_tensor())
            outs = bass2jax._bass_exec_p.bind(
                *operands,
                out_avals=tuple(out_avals),
                in_names=tuple(all_in_names),
                out_names=tuple(out_names),
                lowering_input_output_aliases=(),
                sim_require_finite=True,
                sim_require_nnan=True,
                nc=nc,
            )
            return tuple(outs)

        devices = jax.devices()[:NCORES]
        self.mesh = Mesh(np.asarray(devices), ("core",))
        in_specs = (PartitionSpec("core"),) * (n_params + len(out_names))
        out_specs = (PartitionSpec("core"),) * len(out_names)
        self.fn = jax.jit(
            shard_map(
                _body,
                mesh=self.mesh,
                in_specs=in_specs,
                out_specs=out_specs,
                check_rep=False,
            ),
            donate_argnums=tuple(
                range(n_params, n_params + len(out_names))
            ),
            keep_unused=True,
        )

    def pack(self, inp, action_out, conv_weight):
        """Full inputs -> concatenated per-core arrays (axis 0 = core-major)."""
        x = np.ascontiguousarray(
            np.asarray(inp, dtype=np.float32).reshape(B, C, HW)
        )  # [64,256,4096]; shard_map slices rows 8i:8i+8 per core
        a = np.asarray(action_out, dtype=np.float32).reshape(NCORES, 1, BPC * 9)
        a = np.ascontiguousarray(a.reshape(NCORES * 1, BPC * 9))
        wt = np.asarray(conv_weight, dtype=np.float32).transpose(1, 2, 3, 0)
        wt = wt.reshape(2, 128, 9, C).transpose(1, 2, 0, 3).reshape(128, WCOLS)
        wrep = np.ascontiguousarray(np.broadcast_to(wt, (NCORES, 128, WCOLS)))
        wrep = wrep.reshape(NCORES * 128, WCOLS)
        return [x, a, wrep]

    def zeros(self):
        """Donation buffers for the outputs, created on-device (no transfer)."""
        if not hasattr(self, "_zfn"):
            import jax.numpy as jnp
            from jax.sharding import NamedSharding, PartitionSpec

            sh = NamedSharding(self.mesh, PartitionSpec("core"))
            self._zfn = self.jax.jit(
                lambda: tuple(
                    jnp.zeros((NCORES * av.shape[0], *av.shape[1:]), av.dtype)
                    for av in self.out_avals
                ),
                out_shardings=tuple(sh for _ in self.out_avals),
            )
        return self.jax.block_until_ready(self._zfn())

    def run(self, packed, zeros=None):
        if zeros is None:
            zeros = self.zeros()
        outs = self.fn(*packed, *zeros)
        y = np.asarray(outs[0])
        return y.reshape(B, C, H, W)


def _get_runner():
    global _RUNNER
    if _RUNNER is None:
        _RUNNER = _Runner()
    return _RUNNER


def kernel(inp, action_out, conv_weight):
    r = _get_runner()
    packed = r.pack(inp, action_out, conv_weight)
    return r.run(packed)


if __name__ == "__main__":
    rng = np.random.default_rng(0)
    inp = rng.standard_normal((B, C, H, W), dtype=np.float32)
    act = rng.standard_normal((B, 9), dtype=np.float32)
    wgt = (rng.random((C, C, 3, 3), dtype=np.float32) - 0.5) * (2.0 / (C * 9))
    out = kernel(inp, act, wgt)
    print("out", out.shape, out.dtype, float(np.abs(out).max()))
